# revision 17
# baseline (speedup 1.0000x reference)
"""Causal linear attention (fast-transformers style) on 8 Trainium2 NeuronCores.

Full inputs in, full output out. Sharding: the 32 (n, h) pairs are split
8 ways -> each core owns 4 pairs (one batch n, 4 adjacent heads), so the
per-(n,h) cumulative KV state never crosses cores (no collectives).

Per-core algorithm (chunked scan, chunk C=128 rows):
  phi(x) = elu(x)+1 = max(x,0) + min(exp(x),1)      (exact identity)
  Q = phi(q); K = phi(k) * kl;  V' = [V, 1]          (ones column produces
                                                      the denominator)
  per chunk i, per pair j:
    attn_T[d,c] = sum_e K[d,e] Q[c,e]   masked to d<=c (triu in [d,c])
    out = attn_T^T @ V' + Q @ S         (S = running sum of K^T V' [E, M+1])
    S  += K^T @ V'                      (PSUM accumulation across all chunks)
    result = out[:, :64] / out[:, 64]   (phi>0 so no eps needed; |eps/denom|
                                         would be ~1e-8)

Matmul operands are bf16 (PSUM accumulation stays fp32). Q/K are
transposed to E-major via the DMA xbar. All matmul operands are padded to
K=128 at partition base 0 (zero half-blocks kill the cross terms): this
toolchain's PE crashes (NRT_EXEC_UNIT_UNRECOVERABLE) when consecutive
matmuls' operand base partitions alternate between 0 and 64.

Transposed-block layout (128 cols each) of the phi tile:
  [q0|Z] [q2|Z] [Z|q1] [Z|q3] [k0|k1] [k2|k3]
so qT_j lands on the partition half matching pair parity, kT blocks carry
two pairs. The S state mirrors that parity: pair j at partitions
64*(j%2).., cols 65*(j//2)..
"""

from contextlib import ExitStack

import numpy as np

import concourse.bacc as bacc
import concourse.mybir as mybir
import concourse.tile as tile
from concourse.bass_utils import run_bass_kernel_spmd

F32 = mybir.dt.float32
BF16 = mybir.dt.bfloat16
AF = mybir.ActivationFunctionType
ALU = mybir.AluOpType

N, L, H, E = 4, 4096, 8, 64
P = 4            # (n,h) pairs per core
C = 128          # chunk rows
M1 = E + 1       # v columns + ones column (denominator)
N_CORES = 8
NBUF = 3         # phi ring depth

# col offset of each pair's q data inside the padded q-block region
_QCOL = {0: 0, 2: 128, 1: 320, 3: 448}
# block index of each pair's padded qT block
_QBLK = {0: 0, 2: 1, 1: 2, 3: 3}


def build_core_kernel(nc, seq_len=L):
    """Emit the per-core program. Each core sees [seq_len, P, E] slices."""
    nch = seq_len // C

    q_d = nc.dram_tensor("q", [seq_len, P, E], F32, kind="ExternalInput").ap()
    k_d = nc.dram_tensor("k", [seq_len, P, E], F32, kind="ExternalInput").ap()
    v_d = nc.dram_tensor("v", [seq_len, P, E], F32, kind="ExternalInput").ap()
    kl_d = nc.dram_tensor("kl", [seq_len], F32, kind="ExternalInput").ap()
    tril_d = nc.dram_tensor("tril", [C, P * C], F32, kind="ExternalInput").ap()
    out_d = nc.dram_tensor("out", [seq_len, P, E], F32, kind="ExternalOutput").ap()

    qr = q_d.rearrange("(c p) j e -> c p (j e)", p=C)
    kr = k_d.rearrange("(c p) j e -> c p (j e)", p=C)
    vr = v_d.rearrange("(c p) j e -> c p j e", p=C)
    klr = kl_d.rearrange("(c p) -> p c", p=C)
    outr = out_d.rearrange("(c p) j e -> c p (j e)", p=C)

    with tile.TileContext(nc) as tc, ExitStack() as ctx:
        consts = ctx.enter_context(tc.tile_pool(name="consts", bufs=1))
        qk_pool = ctx.enter_context(tc.tile_pool(name="qk", bufs=NBUF))
        e_pool = ctx.enter_context(tc.tile_pool(name="exp", bufs=NBUF))
        phi_pool = ctx.enter_context(tc.tile_pool(name="phi", bufs=1))
        kn_pool = ctx.enter_context(tc.tile_pool(name="knat", bufs=NBUF))
        vx_pool = ctx.enter_context(tc.tile_pool(name="vx", bufs=NBUF))
        qkT_pool = ctx.enter_context(tc.tile_pool(name="qkT", bufs=NBUF))
        attn_pool = ctx.enter_context(tc.tile_pool(name="attn", bufs=NBUF))
        s_pool = ctx.enter_context(tc.tile_pool(name="ssb", bufs=2))
        z_pool = ctx.enter_context(tc.tile_pool(name="z", bufs=2))
        out_pool = ctx.enter_context(tc.tile_pool(name="osb", bufs=NBUF))
        ps_attn = ctx.enter_context(tc.tile_pool(name="psA", bufs=2, space="PSUM"))
        ps_out = ctx.enter_context(tc.tile_pool(name="psO", bufs=2, space="PSUM"))
        ps_s = ctx.enter_context(tc.tile_pool(name="psS", bufs=1, space="PSUM"))

        tril_t = consts.tile([C, P * C], F32)
        nc.sync.dma_start(out=tril_t[:], in_=tril_d[:])
        kl_t = consts.tile([C, nch], F32)
        nc.sync.dma_start(out=kl_t[:], in_=klr)

        # persistent phi ring: [q0|Z][q2|Z][Z|q1][Z|q3][k0|k1][k2|k3], bf16.
        # The Z half-blocks are zeroed once and never written again.
        phi_bufs = []
        for i in range(NBUF):
            pb = phi_pool.tile([C, 6 * C], BF16, name=f"phib{i}")
            pb3 = pb[:].rearrange("p (b z e) -> p b z e", b=6, z=2)
            nc.gpsimd.memset(pb3[:, 0:2, 1, :], 0.0)  # blocks 0-1 high half
            nc.gpsimd.memset(pb3[:, 2:4, 0, :], 0.0)  # blocks 2-3 low half
            phi_bufs.append(pb)

        # running K^T V' state; pair j at partitions 64*(j%2).., cols
        # 65*(j//2)... Full 512-col row (one bank) keeps partition-offset
        # slices 2KB-aligned for the accumulate bookkeeping.
        s_psum = ps_s.tile([C, 512], F32)

        s_prev = None
        for ci in range(nch):
            # ---- load q,k (fp32): cols [0:256]=q pairs 0-3, [256:512]=k
            qk = qk_pool.tile([C, 2 * P * E], F32)
            nc.sync.dma_start(out=qk[:, 0 : P * E], in_=qr[ci])
            nc.sync.dma_start(out=qk[:, P * E : 2 * P * E], in_=kr[ci])

            # ---- v with ones column, cast to bf16 during DMA (SWDGE)
            vx = vx_pool.tile([C, P * M1], BF16)
            vx3 = vx[:].rearrange("p (j m) -> p j m", j=P)
            nc.gpsimd.memset(vx3[:, :, E : E + 1], 1.0)
            nc.gpsimd.dma_start(out=vx3[:, :, 0:E], in_=vr[ci])

            # ---- phi = max(x,0) + min(exp(x),1), bf16, into padded layout
            et = e_pool.tile([C, 2 * P * E], F32)
            nc.scalar.activation(et[:], qk[:], AF.Exp)
            nc.vector.tensor_scalar_min(et[:], et[:], 1.0)
            phi = phi_bufs[ci % NBUF]
            # q even pairs -> blocks 0-1 low halves; q odd -> blocks 2-3
            # high halves; k -> cols 512:768 contiguous
            pq = phi[:].rearrange("p (a b f) -> p a b f", b=2, f=64)
            qq = qk[:].rearrange("p (a b f) -> p a b f", b=2, f=64)
            eq = et[:].rearrange("p (a b f) -> p a b f", b=2, f=64)
            nc.vector.scalar_tensor_tensor(
                pq[:, 0:2, 0, :], qq[:, 0:2, 0, :], 0.0, eq[:, 0:2, 0, :],
                op0=ALU.max, op1=ALU.add,
            )
            nc.vector.scalar_tensor_tensor(
                pq[:, 2:4, 1, :], qq[:, 0:2, 1, :], 0.0, eq[:, 0:2, 1, :],
                op0=ALU.max, op1=ALU.add,
            )
            nc.vector.scalar_tensor_tensor(
                phi[:, 512:768], qk[:, 256:512], 0.0, et[:, 256:512],
                op0=ALU.max, op1=ALU.add,
            )

            # key_lengths scaling of K for the S-update path (attn path gets
            # it folded into the mask op below)
            knat = kn_pool.tile([C, P * E], BF16)
            nc.vector.tensor_scalar_mul(
                knat[:], phi[:, 512:768], kl_t[:, ci : ci + 1]
            )

            # ---- transpose the 6 blocks to E-major via DMA xbar (bf16)
            qkT = qkT_pool.tile([C, 6 * C], BF16)
            for b in range(6):
                nc.sync.dma_start(
                    out=qkT[:, b * C : (b + 1) * C],
                    in_=phi[:, b * C : (b + 1) * C],
                    transpose=True,
                )

            def qT(j):
                b = _QBLK[j]
                return qkT[:, b * C : (b + 1) * C]

            def kT(j):
                b = 4 + j // 2
                return qkT[:, b * C : (b + 1) * C]

            # ---- attn_T[d, c] = K_d . Q_c  (one PSUM bank for all 4 pairs)
            attn_ps = ps_attn.tile([C, P * C], F32)
            for j in range(P):
                nc.tensor.matmul(
                    attn_ps[:, j * C : (j + 1) * C],
                    kT(j),
                    qT(j),
                    start=(j == 0),
                    stop=(j == P - 1),
                )

            # ---- causal mask (upper-tri in [d, c]) * key_lengths[d], to bf16
            attn_sb = attn_pool.tile([C, P * C], BF16)
            nc.vector.scalar_tensor_tensor(
                attn_sb[:], attn_ps[:], kl_t[:, ci : ci + 1], tril_t[:],
                op0=ALU.mult, op1=ALU.mult,
            )

            # ---- out = attn_T^T @ V' (+ Q @ S_prev)
            out_ps = ps_out.tile([C, P * M1], F32)
            for j in range(P):
                nc.tensor.matmul(
                    out_ps[:, j * M1 : (j + 1) * M1],
                    attn_sb[:, j * C : (j + 1) * C],
                    vx[:, j * M1 : (j + 1) * M1],
                    start=(j == 0),
                    stop=(ci == 0 and j == P - 1),
                )
            if ci > 0:
                for j in range(P):
                    nc.tensor.matmul(
                        out_ps[:, j * M1 : (j + 1) * M1],
                        qT(j),
                        s_prev[:, (j // 2) * M1 : (j // 2 + 1) * M1],
                        start=False,
                        stop=(j == P - 1),
                    )

            # ---- S += K^T @ V' (accumulates in PSUM across all chunks)
            for j in range(P):
                lo = 64 * (j % 2)
                nc.tensor.matmul(
                    s_psum[lo : lo + 64, (j // 2) * M1 : (j // 2 + 1) * M1],
                    knat[:, j * E : (j + 1) * E],  # kl-scaled k_j [128, 64]
                    vx[:, j * M1 : (j + 1) * M1],
                    start=(ci == 0 and j <= 1),
                    stop=(ci == nch - 1 and j >= P - 2),
                    skip_group_check=True,
                )

            # ---- S -> SBUF (bf16) for next chunk's inter term
            if ci < nch - 1:
                s_sb = s_pool.tile([C, (P // 2) * M1], BF16)
                nc.scalar.activation(s_sb[:], s_psum[:, 0 : (P // 2) * M1], AF.Copy)
                s_prev = s_sb

            # ---- normalize: out[:, :64] * 1/denom  (denom = ones-column)
            out3 = out_ps[:].rearrange("p (j m) -> p j m", m=M1)
            zt = z_pool.tile([C, P], F32)
            nc.vector.reciprocal(zt[:], out3[:, :, E])
            osb = out_pool.tile([C, P * E], F32)
            nc.vector.tensor_mul(
                osb[:].rearrange("p (j e) -> p j e", j=P),
                out3[:, :, 0:E],
                zt[:].unsqueeze(2).to_broadcast((C, P, E)),
            )
            nc.sync.dma_start(out=outr[ci], in_=osb[:])

    return nc


def _tril_mask():
    m = np.triu(np.ones((C, C), np.float32))  # keep d<=c in [d,c] layout
    return np.ascontiguousarray(np.tile(m, (1, P)))


_CACHE = {}


def _get_nc():
    if "nc" not in _CACHE:
        nc = bacc.Bacc("TRN2", target_bir_lowering=False, debug=False)
        build_core_kernel(nc)
        nc.compile()
        _CACHE["nc"] = nc
    return _CACHE["nc"]


def kernel(queries, keys, values, key_lengths):
    queries = np.asarray(queries, np.float32)
    keys = np.asarray(keys, np.float32)
    values = np.asarray(values, np.float32)
    key_lengths = np.asarray(key_lengths, np.float32)

    nc = _get_nc()
    tril = _tril_mask()
    in_maps = []
    for c in range(N_CORES):
        n, hg = c // 2, (c % 2) * P
        in_maps.append(
            {
                "q": np.ascontiguousarray(queries[n, :, hg : hg + P, :]),
                "k": np.ascontiguousarray(keys[n, :, hg : hg + P, :]),
                "v": np.ascontiguousarray(values[n, :, hg : hg + P, :]),
                "kl": np.ascontiguousarray(key_lengths[n]),
                "tril": tril,
            }
        )
    res = run_bass_kernel_spmd(nc, in_maps, list(range(N_CORES)))
    out = np.empty((N, L, H, E), np.float32)
    for c, r in enumerate(res.results):
        n, hg = c // 2, (c % 2) * P
        out[n, :, hg : hg + P, :] = r["out"]
    return out


# revision 24
# speedup vs baseline: 1.4266x; 1.4266x over previous
"""Causal linear attention (fast-transformers style) on 8 Trainium2 NeuronCores.

Full inputs in, full output out. Sharding: the 32 (n, h) pairs are split
8 ways -> each core owns 4 pairs (one batch n, 4 adjacent heads), so the
per-(n,h) cumulative KV state never crosses cores (no collectives).

Per-core algorithm (chunked scan, chunk C=128 rows):
  phi(x) = elu(x)+1 = max(x,0) + min(exp(x),1)      (exact identity)
  Q = phi(q); K = phi(k) * kl;  V' = [V, 1]          (ones column produces
                                                      the denominator)
  per chunk i, per pair j:
    attn_T[d,c] = sum_e K[d,e] Q[c,e]   masked to d<=c (triu in [d,c])
    out = attn_T^T @ V' + Q @ S         (S = running sum of K^T V' [E, M+1])
    S  += K^T @ V'                      (PSUM accumulation across all chunks)
    result = out[:, :64] / out[:, 64]   (phi>0 so no eps needed; |eps/denom|
                                         would be ~1e-8)

Matmul operands are bf16 (PSUM accumulation stays fp32). Q/K are
transposed to E-major via the DMA xbar. All matmul operands are padded to
K=128 at partition base 0 (zero half-blocks kill the cross terms): this
toolchain's PE crashes (NRT_EXEC_UNIT_UNRECOVERABLE) when consecutive
matmuls' operand base partitions alternate between 0 and 64.

Transposed-block layout (128 cols each) of the phi tile:
  [q0|Z] [q2|Z] [Z|q1] [Z|q3] [k0|k1] [k2|k3]
so qT_j lands on the partition half matching pair parity, kT blocks carry
two pairs. The S state mirrors that parity: pair j at partitions
64*(j%2).., cols 65*(j//2)..
"""

from contextlib import ExitStack

import ml_dtypes
import numpy as np

import concourse.bacc as bacc
import concourse.mybir as mybir
import concourse.tile as tile
from concourse.bass_utils import run_bass_kernel_spmd

F32 = mybir.dt.float32
BF16 = mybir.dt.bfloat16
AF = mybir.ActivationFunctionType
ALU = mybir.AluOpType

N, L, H, E = 4, 4096, 8, 64
P = 4            # (n,h) pairs per core
C = 128          # chunk rows
M1 = E + 1       # v columns + ones column (denominator)
N_CORES = 8
NBUF = 3         # phi ring depth

# col offset of each pair's q data inside the padded q-block region
_QCOL = {0: 0, 2: 128, 1: 320, 3: 448}
# block index of each pair's padded qT block
_QBLK = {0: 0, 2: 1, 1: 2, 3: 3}


def build_core_kernel(nc, seq_len=L):
    """Emit the per-core program. Each core sees [seq_len, P, E] slices."""
    nch = seq_len // C

    q_d = nc.dram_tensor("q", [seq_len, P, E], F32, kind="ExternalInput").ap()
    k_d = nc.dram_tensor("k", [seq_len, P, E], F32, kind="ExternalInput").ap()
    v_d = nc.dram_tensor("v", [seq_len, P, E], F32, kind="ExternalInput").ap()
    kl_d = nc.dram_tensor("kl", [seq_len], F32, kind="ExternalInput").ap()
    tril_d = nc.dram_tensor("tril", [C, P * C], F32, kind="ExternalInput").ap()
    ident_d = nc.dram_tensor("ident", [C, C], BF16, kind="ExternalInput").ap()
    out_d = nc.dram_tensor("out", [seq_len, P, E], F32, kind="ExternalOutput").ap()

    qr = q_d.rearrange("(c p) j e -> c p (j e)", p=C)
    kr = k_d.rearrange("(c p) j e -> c p (j e)", p=C)
    vr = v_d.rearrange("(c p) j e -> c p j e", p=C)
    klr = kl_d.rearrange("(c p) -> p c", p=C)
    outr = out_d.rearrange("(c p) j e -> c p (j e)", p=C)

    with tile.TileContext(nc) as tc, ExitStack() as ctx:
        consts = ctx.enter_context(tc.tile_pool(name="consts", bufs=1))
        qk_pool = ctx.enter_context(tc.tile_pool(name="qk", bufs=NBUF))
        e_pool = ctx.enter_context(tc.tile_pool(name="exp", bufs=NBUF))
        phi_pool = ctx.enter_context(tc.tile_pool(name="phi", bufs=1))
        kn_pool = ctx.enter_context(tc.tile_pool(name="knat", bufs=NBUF))
        vx_pool = ctx.enter_context(tc.tile_pool(name="vx", bufs=NBUF))
        qkT_pool = ctx.enter_context(tc.tile_pool(name="qkT", bufs=NBUF))
        attn_pool = ctx.enter_context(tc.tile_pool(name="attn", bufs=NBUF))
        s_pool = ctx.enter_context(tc.tile_pool(name="ssb", bufs=2))
        z_pool = ctx.enter_context(tc.tile_pool(name="z", bufs=2))
        out_pool = ctx.enter_context(tc.tile_pool(name="osb", bufs=NBUF))
        ps_attn = ctx.enter_context(tc.tile_pool(name="psA", bufs=1, space="PSUM"))
        ps_out = ctx.enter_context(tc.tile_pool(name="psO", bufs=2, space="PSUM"))
        ps_s = ctx.enter_context(tc.tile_pool(name="psS", bufs=1, space="PSUM"))
        ps_tq = ctx.enter_context(tc.tile_pool(name="psTq", bufs=2, space="PSUM"))
        ps_tk = ctx.enter_context(tc.tile_pool(name="psTk", bufs=2, space="PSUM"))

        tril_t = consts.tile([C, P * C], F32)
        nc.sync.dma_start(out=tril_t[:], in_=tril_d[:])
        kl_t = consts.tile([C, nch], F32)
        nc.sync.dma_start(out=kl_t[:], in_=klr)
        ident = consts.tile([C, C], BF16)
        nc.sync.dma_start(out=ident[:], in_=ident_d[:])

        # persistent phi ring: [q0|Z][q2|Z][Z|q1][Z|q3][k0|k1][k2|k3], bf16.
        # The Z half-blocks are zeroed once and never written again.
        phi_bufs = []
        for i in range(NBUF):
            pb = phi_pool.tile([C, 6 * C], BF16, name=f"phib{i}")
            pb3 = pb[:].rearrange("p (b z e) -> p b z e", b=6, z=2)
            nc.gpsimd.memset(pb3[:, 0:2, 1, :], 0.0)  # blocks 0-1 high half
            nc.gpsimd.memset(pb3[:, 2:4, 0, :], 0.0)  # blocks 2-3 low half
            phi_bufs.append(pb)

        # running K^T V' state; pair j at partitions 64*(j%2).., cols
        # 65*(j//2)... Full 512-col row (one bank) keeps partition-offset
        # slices 2KB-aligned for the accumulate bookkeeping.
        s_psum = ps_s.tile([C, 512], F32)

        s_prev = None
        for ci in range(nch):
            # ---- load q,k (fp32): cols [0:256]=q pairs 0-3, [256:512]=k
            # (q on the SP HWDGE ring, k on the ACT ring)
            qk = qk_pool.tile([C, 2 * P * E], F32)
            nc.sync.dma_start(out=qk[:, 0 : P * E], in_=qr[ci])
            nc.scalar.dma_start(out=qk[:, P * E : 2 * P * E], in_=kr[ci])

            # ---- v with ones column, cast to bf16 during DMA (SWDGE)
            vx = vx_pool.tile([C, P * M1], BF16)
            vx3 = vx[:].rearrange("p (j m) -> p j m", j=P)
            nc.gpsimd.memset(vx3[:, :, E : E + 1], 1.0)
            nc.gpsimd.dma_start(out=vx3[:, :, 0:E], in_=vr[ci])

            # ---- phi = max(x,0) + min(exp(x),1), bf16, into padded layout
            et = e_pool.tile([C, 2 * P * E], F32)
            nc.scalar.activation(et[:], qk[:], AF.Exp)
            nc.gpsimd.tensor_scalar_min(et[:], et[:], 1.0)
            phi = phi_bufs[ci % NBUF]
            # q even pairs -> blocks 0-1 low halves; q odd -> blocks 2-3
            # high halves; k -> cols 512:768 contiguous
            pq = phi[:].rearrange("p (a b f) -> p a b f", b=2, f=64)
            qq = qk[:].rearrange("p (a b f) -> p a b f", b=2, f=64)
            eq = et[:].rearrange("p (a b f) -> p a b f", b=2, f=64)
            nc.vector.scalar_tensor_tensor(
                pq[:, 0:2, 0, :], qq[:, 0:2, 0, :], 0.0, eq[:, 0:2, 0, :],
                op0=ALU.max, op1=ALU.add,
            )
            nc.vector.scalar_tensor_tensor(
                pq[:, 2:4, 1, :], qq[:, 0:2, 1, :], 0.0, eq[:, 0:2, 1, :],
                op0=ALU.max, op1=ALU.add,
            )
            nc.vector.scalar_tensor_tensor(
                phi[:, 512:768], qk[:, 256:512], 0.0, et[:, 256:512],
                op0=ALU.max, op1=ALU.add,
            )

            # key_lengths scaling of K for the S-update path (attn path gets
            # it folded into the mask op below)
            knat = kn_pool.tile([C, P * E], BF16)
            nc.vector.tensor_scalar_mul(
                knat[:], phi[:, 512:768], kl_t[:, ci : ci + 1]
            )

            # ---- transpose the 6 blocks to E-major via PE identity matmuls
            # (out = phi_block^T @ I); the HWDGE rings can't absorb xbar
            # transposes (1.2us each, FIFO per ring)
            tq_ps = ps_tq.tile([C, 4 * C], F32)
            tk_ps = ps_tk.tile([C, 2 * C], F32)
            for b in range(4):
                nc.tensor.matmul(
                    tq_ps[:, b * C : (b + 1) * C],
                    phi[:, b * C : (b + 1) * C],
                    ident[:],
                    start=(b == 0),
                    stop=(b == 3),
                )
            for b in range(2):
                nc.tensor.matmul(
                    tk_ps[:, b * C : (b + 1) * C],
                    phi[:, (4 + b) * C : (5 + b) * C],
                    ident[:],
                    start=(b == 0),
                    stop=(b == 1),
                )
            qkT = qkT_pool.tile([C, 6 * C], BF16)
            nc.scalar.activation(qkT[:, 0 : 4 * C], tq_ps[:], AF.Copy)
            nc.vector.tensor_copy(qkT[:, 4 * C : 6 * C], tk_ps[:])

            def qT(j):
                b = _QBLK[j]
                return qkT[:, b * C : (b + 1) * C]

            def kT(j):
                b = 4 + j // 2
                return qkT[:, b * C : (b + 1) * C]

            # ---- attn_T[d, c] = K_d . Q_c  (one PSUM bank for all 4 pairs)
            attn_ps = ps_attn.tile([C, P * C], F32)
            for j in range(P):
                nc.tensor.matmul(
                    attn_ps[:, j * C : (j + 1) * C],
                    kT(j),
                    qT(j),
                    start=(j == 0),
                    stop=(j == P - 1),
                )

            # ---- causal mask (upper-tri in [d, c]) * key_lengths[d], to bf16
            attn_sb = attn_pool.tile([C, P * C], BF16)
            nc.vector.scalar_tensor_tensor(
                attn_sb[:], attn_ps[:], kl_t[:, ci : ci + 1], tril_t[:],
                op0=ALU.mult, op1=ALU.mult,
            )

            # ---- out = attn_T^T @ V' (+ Q @ S_prev)
            out_ps = ps_out.tile([C, P * M1], F32)
            for j in range(P):
                nc.tensor.matmul(
                    out_ps[:, j * M1 : (j + 1) * M1],
                    attn_sb[:, j * C : (j + 1) * C],
                    vx[:, j * M1 : (j + 1) * M1],
                    start=(j == 0),
                    stop=(ci == 0 and j == P - 1),
                )
            if ci > 0:
                for j in range(P):
                    nc.tensor.matmul(
                        out_ps[:, j * M1 : (j + 1) * M1],
                        qT(j),
                        s_prev[:, (j // 2) * M1 : (j // 2 + 1) * M1],
                        start=False,
                        stop=(j == P - 1),
                    )

            # ---- S += K^T @ V' (accumulates in PSUM across all chunks)
            for j in range(P):
                lo = 64 * (j % 2)
                nc.tensor.matmul(
                    s_psum[lo : lo + 64, (j // 2) * M1 : (j // 2 + 1) * M1],
                    knat[:, j * E : (j + 1) * E],  # kl-scaled k_j [128, 64]
                    vx[:, j * M1 : (j + 1) * M1],
                    start=(ci == 0 and j <= 1),
                    stop=(ci == nch - 1 and j >= P - 2),
                    skip_group_check=True,
                )

            # ---- S -> SBUF (bf16) for next chunk's inter term
            if ci < nch - 1:
                s_sb = s_pool.tile([C, (P // 2) * M1], BF16)
                nc.scalar.activation(s_sb[:], s_psum[:, 0 : (P // 2) * M1], AF.Copy)
                s_prev = s_sb

            # ---- normalize: out[:, :64] * 1/denom  (denom = ones-column)
            out3 = out_ps[:].rearrange("p (j m) -> p j m", m=M1)
            zt = z_pool.tile([C, P], F32)
            nc.vector.reciprocal(zt[:], out3[:, :, E])
            osb = out_pool.tile([C, P * E], F32)
            nc.vector.tensor_mul(
                osb[:].rearrange("p (j e) -> p j e", j=P),
                out3[:, :, 0:E],
                zt[:].unsqueeze(2).to_broadcast((C, P, E)),
            )
            nc.sync.dma_start(out=outr[ci], in_=osb[:])

    return nc


def _tril_mask():
    m = np.triu(np.ones((C, C), np.float32))  # keep d<=c in [d,c] layout
    return np.ascontiguousarray(np.tile(m, (1, P)))


def _ident_bf16():
    return np.eye(C, dtype=ml_dtypes.bfloat16)


_CACHE = {}


def _get_nc():
    if "nc" not in _CACHE:
        nc = bacc.Bacc("TRN2", target_bir_lowering=False, debug=False)
        build_core_kernel(nc)
        nc.compile()
        _CACHE["nc"] = nc
    return _CACHE["nc"]


def kernel(queries, keys, values, key_lengths):
    queries = np.asarray(queries, np.float32)
    keys = np.asarray(keys, np.float32)
    values = np.asarray(values, np.float32)
    key_lengths = np.asarray(key_lengths, np.float32)

    nc = _get_nc()
    tril = _tril_mask()
    in_maps = []
    for c in range(N_CORES):
        n, hg = c // 2, (c % 2) * P
        in_maps.append(
            {
                "q": np.ascontiguousarray(queries[n, :, hg : hg + P, :]),
                "k": np.ascontiguousarray(keys[n, :, hg : hg + P, :]),
                "v": np.ascontiguousarray(values[n, :, hg : hg + P, :]),
                "kl": np.ascontiguousarray(key_lengths[n]),
                "tril": tril,
                "ident": _ident_bf16(),
            }
        )
    res = run_bass_kernel_spmd(nc, in_maps, list(range(N_CORES)))
    out = np.empty((N, L, H, E), np.float32)
    for c, r in enumerate(res.results):
        n, hg = c // 2, (c % 2) * P
        out[n, :, hg : hg + P, :] = r["out"]
    return out


# revision 25
# speedup vs baseline: 3.7916x; 2.6579x over previous
"""Causal linear attention (fast-transformers style) on 8 Trainium2 NeuronCores.

Full inputs in, full output out. Sharding: the 32 (n, h) pairs are split
8 ways -> each core owns 4 pairs (one batch n, 4 adjacent heads), so the
per-(n,h) cumulative KV state never crosses cores (no collectives).

Per-core algorithm (chunked scan, chunk C=128 rows):
  phi(x) = elu(x)+1 = max(x,0) + min(exp(x),1)      (exact identity)
  Q = phi(q); K = phi(k) * kl;  V' = [V, 1]          (ones column produces
                                                      the denominator)
  per chunk i, per pair j:
    attn_T[d,c] = sum_e K[d,e] Q[c,e]   masked to d<=c (triu in [d,c])
    out = attn_T^T @ V' + Q @ S         (S = running sum of K^T V' [E, M+1])
    S  += K^T @ V'                      (PSUM accumulation across all chunks)
    result = out[:, :64] / out[:, 64]   (phi>0 so no eps needed; |eps/denom|
                                         would be ~1e-8)

Matmul operands are bf16 (PSUM accumulation stays fp32). Q/K are
transposed to E-major via the DMA xbar. All matmul operands are padded to
K=128 at partition base 0 (zero half-blocks kill the cross terms): this
toolchain's PE crashes (NRT_EXEC_UNIT_UNRECOVERABLE) when consecutive
matmuls' operand base partitions alternate between 0 and 64.

Transposed-block layout (128 cols each) of the phi tile:
  [q0|Z] [q2|Z] [Z|q1] [Z|q3] [k0|k1] [k2|k3]
so qT_j lands on the partition half matching pair parity, kT blocks carry
two pairs. The S state mirrors that parity: pair j at partitions
64*(j%2).., cols 65*(j//2)..
"""

from contextlib import ExitStack

import ml_dtypes
import numpy as np

import concourse.bacc as bacc
import concourse.mybir as mybir
import concourse.tile as tile
from concourse.bass_utils import run_bass_kernel_spmd

F32 = mybir.dt.float32
BF16 = mybir.dt.bfloat16
AF = mybir.ActivationFunctionType
ALU = mybir.AluOpType

N, L, H, E = 4, 4096, 8, 64
P = 4            # (n,h) pairs per core
C = 128          # chunk rows
M1 = E + 1       # v columns + ones column (denominator)
N_CORES = 8
NBUF = 3         # phi ring depth

# col offset of each pair's q data inside the padded q-block region
_QCOL = {0: 0, 2: 128, 1: 320, 3: 448}
# block index of each pair's padded qT block
_QBLK = {0: 0, 2: 1, 1: 2, 3: 3}


def build_core_kernel(nc, seq_len=L):
    """Emit the per-core program. Each core sees [seq_len, P, E] slices."""
    nch = seq_len // C

    q_d = nc.dram_tensor("q", [seq_len, P, E], F32, kind="ExternalInput").ap()
    k_d = nc.dram_tensor("k", [seq_len, P, E], F32, kind="ExternalInput").ap()
    v_d = nc.dram_tensor("v", [seq_len, P, E], F32, kind="ExternalInput").ap()
    kl_d = nc.dram_tensor("kl", [seq_len], F32, kind="ExternalInput").ap()
    tril_d = nc.dram_tensor("tril", [C, P * C], F32, kind="ExternalInput").ap()
    ident_d = nc.dram_tensor("ident", [C, C], BF16, kind="ExternalInput").ap()
    out_d = nc.dram_tensor("out", [seq_len, P, E], F32, kind="ExternalOutput").ap()

    qr = q_d.rearrange("(c p) j e -> c p (j e)", p=C)
    kr = k_d.rearrange("(c p) j e -> c p (j e)", p=C)
    vr = v_d.rearrange("(c p) j e -> c p j e", p=C)
    klr = kl_d.rearrange("(c p) -> p c", p=C)
    outr = out_d.rearrange("(c p) j e -> c p (j e)", p=C)

    with tile.TileContext(nc) as tc, ExitStack() as ctx:
        consts = ctx.enter_context(tc.tile_pool(name="consts", bufs=1))
        qk_pool = ctx.enter_context(tc.tile_pool(name="qk", bufs=NBUF))
        e_pool = ctx.enter_context(tc.tile_pool(name="exp", bufs=NBUF))
        phi_pool = ctx.enter_context(tc.tile_pool(name="phi", bufs=1))
        kn_pool = ctx.enter_context(tc.tile_pool(name="knat", bufs=NBUF))
        vx_pool = ctx.enter_context(tc.tile_pool(name="vx", bufs=NBUF))
        qkT_pool = ctx.enter_context(tc.tile_pool(name="qkT", bufs=NBUF))
        attn_pool = ctx.enter_context(tc.tile_pool(name="attn", bufs=NBUF))
        s_pool = ctx.enter_context(tc.tile_pool(name="ssb", bufs=2))
        z_pool = ctx.enter_context(tc.tile_pool(name="z", bufs=2))
        out_pool = ctx.enter_context(tc.tile_pool(name="osb", bufs=NBUF))
        ps_attn = ctx.enter_context(tc.tile_pool(name="psA", bufs=1, space="PSUM"))
        ps_out = ctx.enter_context(tc.tile_pool(name="psO", bufs=2, space="PSUM"))
        ps_s = ctx.enter_context(tc.tile_pool(name="psS", bufs=1, space="PSUM"))
        ps_tq = ctx.enter_context(tc.tile_pool(name="psTq", bufs=2, space="PSUM"))
        ps_tk = ctx.enter_context(tc.tile_pool(name="psTk", bufs=2, space="PSUM"))

        tril_t = consts.tile([C, P * C], F32)
        nc.sync.dma_start(out=tril_t[:], in_=tril_d[:])
        kl_t = consts.tile([C, nch], F32)
        nc.sync.dma_start(out=kl_t[:], in_=klr)
        ident = consts.tile([C, C], BF16)
        nc.sync.dma_start(out=ident[:], in_=ident_d[:])

        # persistent phi ring: [q0|Z][q2|Z][Z|q1][Z|q3][k0|k1][k2|k3], bf16.
        # The Z half-blocks are zeroed once and never written again.
        phi_bufs = []
        for i in range(NBUF):
            pb = phi_pool.tile([C, 6 * C], BF16, name=f"phib{i}")
            pb3 = pb[:].rearrange("p (b z e) -> p b z e", b=6, z=2)
            nc.gpsimd.memset(pb3[:, 0:2, 1, :], 0.0)  # blocks 0-1 high half
            nc.gpsimd.memset(pb3[:, 2:4, 0, :], 0.0)  # blocks 2-3 low half
            phi_bufs.append(pb)

        # running K^T V' state; pair j at partitions 64*(j%2).., cols
        # 65*(j//2)... Full 512-col row (one bank) keeps partition-offset
        # slices 2KB-aligned for the accumulate bookkeeping.
        s_psum = ps_s.tile([C, 512], F32)

        s_prev = None
        for ci in range(nch):
            # ---- load q,k (fp32): cols [0:256]=q pairs 0-3, [256:512]=k
            # (q on the SP HWDGE ring, k on the ACT ring)
            qk = qk_pool.tile([C, 2 * P * E], F32)
            nc.sync.dma_start(out=qk[:, 0 : P * E], in_=qr[ci])
            nc.scalar.dma_start(out=qk[:, P * E : 2 * P * E], in_=kr[ci])

            # ---- v with ones column, cast to bf16 during DMA (SWDGE)
            vx = vx_pool.tile([C, P * M1], BF16)
            vx3 = vx[:].rearrange("p (j m) -> p j m", j=P)
            nc.gpsimd.memset(vx3[:, :, E : E + 1], 1.0)
            nc.gpsimd.dma_start(out=vx3[:, :, 0:E], in_=vr[ci])

            # ---- phi = max(x,0) + min(exp(x),1), bf16, into padded layout
            et = e_pool.tile([C, 2 * P * E], F32)
            nc.scalar.activation(et[:], qk[:], AF.Exp)
            nc.vector.tensor_scalar_min(et[:], et[:], 1.0)
            phi = phi_bufs[ci % NBUF]
            # q even pairs -> blocks 0-1 low halves; q odd -> blocks 2-3
            # high halves; k -> cols 512:768 contiguous
            pq = phi[:].rearrange("p (a b f) -> p a b f", b=2, f=64)
            qq = qk[:].rearrange("p (a b f) -> p a b f", b=2, f=64)
            eq = et[:].rearrange("p (a b f) -> p a b f", b=2, f=64)
            nc.vector.scalar_tensor_tensor(
                pq[:, 0:2, 0, :], qq[:, 0:2, 0, :], 0.0, eq[:, 0:2, 0, :],
                op0=ALU.max, op1=ALU.add,
            )
            nc.vector.scalar_tensor_tensor(
                pq[:, 2:4, 1, :], qq[:, 0:2, 1, :], 0.0, eq[:, 0:2, 1, :],
                op0=ALU.max, op1=ALU.add,
            )
            nc.vector.scalar_tensor_tensor(
                phi[:, 512:768], qk[:, 256:512], 0.0, et[:, 256:512],
                op0=ALU.max, op1=ALU.add,
            )

            # key_lengths scaling of K for the S-update path (attn path gets
            # it folded into the mask op below)
            knat = kn_pool.tile([C, P * E], BF16)
            nc.vector.tensor_scalar_mul(
                knat[:], phi[:, 512:768], kl_t[:, ci : ci + 1]
            )

            # ---- transpose the 6 blocks to E-major via PE identity matmuls
            # (out = phi_block^T @ I); the HWDGE rings can't absorb xbar
            # transposes (1.2us each, FIFO per ring)
            tq_ps = ps_tq.tile([C, 4 * C], F32)
            tk_ps = ps_tk.tile([C, 2 * C], F32)
            for b in range(4):
                nc.tensor.matmul(
                    tq_ps[:, b * C : (b + 1) * C],
                    phi[:, b * C : (b + 1) * C],
                    ident[:],
                    start=(b == 0),
                    stop=(b == 3),
                )
            for b in range(2):
                nc.tensor.matmul(
                    tk_ps[:, b * C : (b + 1) * C],
                    phi[:, (4 + b) * C : (5 + b) * C],
                    ident[:],
                    start=(b == 0),
                    stop=(b == 1),
                )
            qkT = qkT_pool.tile([C, 6 * C], BF16)
            nc.scalar.activation(qkT[:, 0 : 4 * C], tq_ps[:], AF.Copy)
            nc.vector.tensor_copy(qkT[:, 4 * C : 6 * C], tk_ps[:])

            def qT(j):
                b = _QBLK[j]
                return qkT[:, b * C : (b + 1) * C]

            def kT(j):
                b = 4 + j // 2
                return qkT[:, b * C : (b + 1) * C]

            # ---- attn_T[d, c] = K_d . Q_c  (one PSUM bank for all 4 pairs)
            attn_ps = ps_attn.tile([C, P * C], F32)
            for j in range(P):
                nc.tensor.matmul(
                    attn_ps[:, j * C : (j + 1) * C],
                    kT(j),
                    qT(j),
                    start=(j == 0),
                    stop=(j == P - 1),
                )

            # ---- causal mask (upper-tri in [d, c]) * key_lengths[d], to bf16
            attn_sb = attn_pool.tile([C, P * C], BF16)
            nc.vector.scalar_tensor_tensor(
                attn_sb[:], attn_ps[:], kl_t[:, ci : ci + 1], tril_t[:],
                op0=ALU.mult, op1=ALU.mult,
            )

            # ---- out = attn_T^T @ V' (+ Q @ S_prev)
            out_ps = ps_out.tile([C, P * M1], F32)
            for j in range(P):
                nc.tensor.matmul(
                    out_ps[:, j * M1 : (j + 1) * M1],
                    attn_sb[:, j * C : (j + 1) * C],
                    vx[:, j * M1 : (j + 1) * M1],
                    start=(j == 0),
                    stop=(ci == 0 and j == P - 1),
                )
            if ci > 0:
                for j in range(P):
                    nc.tensor.matmul(
                        out_ps[:, j * M1 : (j + 1) * M1],
                        qT(j),
                        s_prev[:, (j // 2) * M1 : (j // 2 + 1) * M1],
                        start=False,
                        stop=(j == P - 1),
                    )

            # ---- S += K^T @ V' (accumulates in PSUM across all chunks)
            for j in range(P):
                lo = 64 * (j % 2)
                nc.tensor.matmul(
                    s_psum[lo : lo + 64, (j // 2) * M1 : (j // 2 + 1) * M1],
                    knat[:, j * E : (j + 1) * E],  # kl-scaled k_j [128, 64]
                    vx[:, j * M1 : (j + 1) * M1],
                    start=(ci == 0 and j <= 1),
                    stop=(ci == nch - 1 and j >= P - 2),
                    skip_group_check=True,
                )

            # ---- S -> SBUF (bf16) for next chunk's inter term
            if ci < nch - 1:
                s_sb = s_pool.tile([C, (P // 2) * M1], BF16)
                nc.scalar.activation(s_sb[:], s_psum[:, 0 : (P // 2) * M1], AF.Copy)
                s_prev = s_sb

            # ---- normalize: out[:, :64] * 1/denom  (denom = ones-column)
            out3 = out_ps[:].rearrange("p (j m) -> p j m", m=M1)
            zt = z_pool.tile([C, P], F32)
            nc.vector.reciprocal(zt[:], out3[:, :, E])
            osb = out_pool.tile([C, P * E], F32)
            nc.vector.tensor_mul(
                osb[:].rearrange("p (j e) -> p j e", j=P),
                out3[:, :, 0:E],
                zt[:].unsqueeze(2).to_broadcast((C, P, E)),
            )
            nc.sync.dma_start(out=outr[ci], in_=osb[:])

    return nc


def _tril_mask():
    m = np.triu(np.ones((C, C), np.float32))  # keep d<=c in [d,c] layout
    return np.ascontiguousarray(np.tile(m, (1, P)))


def _ident_bf16():
    return np.eye(C, dtype=ml_dtypes.bfloat16)


_CACHE = {}


def _get_nc():
    if "nc" not in _CACHE:
        nc = bacc.Bacc("TRN2", target_bir_lowering=False, debug=False)
        build_core_kernel(nc)
        nc.compile()
        _CACHE["nc"] = nc
    return _CACHE["nc"]


def kernel(queries, keys, values, key_lengths):
    queries = np.asarray(queries, np.float32)
    keys = np.asarray(keys, np.float32)
    values = np.asarray(values, np.float32)
    key_lengths = np.asarray(key_lengths, np.float32)

    nc = _get_nc()
    tril = _tril_mask()
    in_maps = []
    for c in range(N_CORES):
        n, hg = c // 2, (c % 2) * P
        in_maps.append(
            {
                "q": np.ascontiguousarray(queries[n, :, hg : hg + P, :]),
                "k": np.ascontiguousarray(keys[n, :, hg : hg + P, :]),
                "v": np.ascontiguousarray(values[n, :, hg : hg + P, :]),
                "kl": np.ascontiguousarray(key_lengths[n]),
                "tril": tril,
                "ident": _ident_bf16(),
            }
        )
    res = run_bass_kernel_spmd(nc, in_maps, list(range(N_CORES)))
    out = np.empty((N, L, H, E), np.float32)
    for c, r in enumerate(res.results):
        n, hg = c // 2, (c % 2) * P
        out[n, :, hg : hg + P, :] = r["out"]
    return out


# revision 28
# speedup vs baseline: 3.8544x; 1.0165x over previous
"""Causal linear attention (fast-transformers style) on 8 Trainium2 NeuronCores.

Full inputs in, full output out. Sharding: the 32 (n, h) pairs are split
8 ways -> each core owns 4 pairs (one batch n, 4 adjacent heads), so the
per-(n,h) cumulative KV state never crosses cores (no collectives).

Per-core algorithm (chunked scan, chunk C=128 rows):
  phi(x) = elu(x)+1 = max(x,0) + min(exp(x),1)      (exact identity)
  Q = phi(q); K = phi(k) * kl;  V' = [V, 1]          (ones column produces
                                                      the denominator)
  per chunk i, per pair j:
    attn_T[d,c] = sum_e K[d,e] Q[c,e]   masked to d<=c (triu in [d,c])
    out = attn_T^T @ V' + Q @ S         (S = running sum of K^T V' [E, M+1])
    S  += K^T @ V'                      (PSUM accumulation across all chunks)
    result = out[:, :64] / out[:, 64]   (phi>0 so no eps needed; |eps/denom|
                                         would be ~1e-8)

Matmul operands are bf16 (PSUM accumulation stays fp32). Q/K are
transposed to E-major via the DMA xbar. All matmul operands are padded to
K=128 at partition base 0 (zero half-blocks kill the cross terms): this
toolchain's PE crashes (NRT_EXEC_UNIT_UNRECOVERABLE) when consecutive
matmuls' operand base partitions alternate between 0 and 64.

Transposed-block layout (128 cols each) of the phi tile:
  [q0|Z] [q2|Z] [Z|q1] [Z|q3] [k0|k1] [k2|k3]
so qT_j lands on the partition half matching pair parity, kT blocks carry
two pairs. The S state mirrors that parity: pair j at partitions
64*(j%2).., cols 65*(j//2)..
"""

from contextlib import ExitStack

import ml_dtypes
import numpy as np

import concourse.bacc as bacc
import concourse.mybir as mybir
import concourse.tile as tile
from concourse.bass_utils import run_bass_kernel_spmd

F32 = mybir.dt.float32
BF16 = mybir.dt.bfloat16
AF = mybir.ActivationFunctionType
ALU = mybir.AluOpType

N, L, H, E = 4, 4096, 8, 64
P = 4            # (n,h) pairs per core
C = 128          # chunk rows
M1 = E + 1       # v columns + ones column (denominator)
N_CORES = 8
NBUF = 3         # phi ring depth

# col offset of each pair's q data inside the padded q-block region
_QCOL = {0: 0, 2: 128, 1: 320, 3: 448}
# block index of each pair's padded qT block
_QBLK = {0: 0, 2: 1, 1: 2, 3: 3}


def build_core_kernel(nc, seq_len=L):
    """Emit the per-core program. Each core sees [seq_len, P, E] slices."""
    nch = seq_len // C

    q_d = nc.dram_tensor("q", [seq_len, P, E], F32, kind="ExternalInput").ap()
    k_d = nc.dram_tensor("k", [seq_len, P, E], F32, kind="ExternalInput").ap()
    v_d = nc.dram_tensor("v", [seq_len, P, E], F32, kind="ExternalInput").ap()
    kl_d = nc.dram_tensor("kl", [seq_len], F32, kind="ExternalInput").ap()
    tril_d = nc.dram_tensor("tril", [C, P * C], F32, kind="ExternalInput").ap()
    ident_d = nc.dram_tensor("ident", [C, C], BF16, kind="ExternalInput").ap()
    out_d = nc.dram_tensor("out", [seq_len, P, E], F32, kind="ExternalOutput").ap()

    qr = q_d.rearrange("(c p) j e -> c p (j e)", p=C)
    kr = k_d.rearrange("(c p) j e -> c p (j e)", p=C)
    qr2 = q_d.rearrange("(c t p) j e -> c p t (j e)", t=2, p=C)
    kr2 = k_d.rearrange("(c t p) j e -> c p t (j e)", t=2, p=C)
    vr = v_d.rearrange("(c p) j e -> c p j e", p=C)
    klr = kl_d.rearrange("(c p) -> p c", p=C)
    outr = out_d.rearrange("(c p) j e -> c p (j e)", p=C)
    outr2 = out_d.rearrange("(c t p) j e -> c p t (j e)", t=2, p=C)

    with tile.TileContext(nc) as tc, ExitStack() as ctx:
        consts = ctx.enter_context(tc.tile_pool(name="consts", bufs=1))
        qk_pool = ctx.enter_context(tc.tile_pool(name="qk", bufs=NBUF))
        e_pool = ctx.enter_context(tc.tile_pool(name="exp", bufs=2))
        x_pool = ctx.enter_context(tc.tile_pool(name="xmax", bufs=2))
        phi_pool = ctx.enter_context(tc.tile_pool(name="phi", bufs=1))
        vx_pool = ctx.enter_context(tc.tile_pool(name="vx", bufs=NBUF))
        qkT_pool = ctx.enter_context(tc.tile_pool(name="qkT", bufs=NBUF))
        attn_pool = ctx.enter_context(tc.tile_pool(name="attn", bufs=NBUF))
        s_pool = ctx.enter_context(tc.tile_pool(name="ssb", bufs=2))
        z_pool = ctx.enter_context(tc.tile_pool(name="z", bufs=2))
        out_pool = ctx.enter_context(tc.tile_pool(name="osb", bufs=NBUF))
        ps_attn = ctx.enter_context(tc.tile_pool(name="psA", bufs=1, space="PSUM"))
        ps_out = ctx.enter_context(tc.tile_pool(name="psO", bufs=2, space="PSUM"))
        ps_s = ctx.enter_context(tc.tile_pool(name="psS", bufs=1, space="PSUM"))
        ps_tq = ctx.enter_context(tc.tile_pool(name="psTq", bufs=2, space="PSUM"))
        ps_tk = ctx.enter_context(tc.tile_pool(name="psTk", bufs=2, space="PSUM"))

        tril_t = consts.tile([C, P * C], F32)
        nc.sync.dma_start(out=tril_t[:], in_=tril_d[:])
        kl_t = consts.tile([C, nch], F32)
        nc.sync.dma_start(out=kl_t[:], in_=klr)
        ident = consts.tile([C, C], BF16)
        nc.sync.dma_start(out=ident[:], in_=ident_d[:])

        # persistent phi ring: [q0|Z][q2|Z][Z|q1][Z|q3][k0|k1][k2|k3], bf16.
        # The Z half-blocks are zeroed once and never written again.
        phi_bufs = []
        for i in range(NBUF):
            pb = phi_pool.tile([C, 6 * C], BF16, name=f"phib{i}")
            pb3 = pb[:].rearrange("p (b z e) -> p b z e", b=6, z=2)
            nc.gpsimd.memset(pb3[:, 0:2, 1, :], 0.0)  # blocks 0-1 high half
            nc.gpsimd.memset(pb3[:, 2:4, 0, :], 0.0)  # blocks 2-3 low half
            phi_bufs.append(pb)

        # running K^T V' state; pair j at partitions 64*(j%2).., cols
        # 65*(j//2)... Full 512-col row (one bank) keeps partition-offset
        # slices 2KB-aligned for the accumulate bookkeeping.
        s_psum = ps_s.tile([C, 512], F32)

        assert nch % 2 == 0
        s_prev = None
        for ci in range(nch):
            cb, c2 = ci // 2, ci % 2
            if c2 == 0:
                # ---- load q,k for TWO chunks (fp32): layout
                # [q(2c) | q(2c+1) | k(2c) | k(2c+1)], 256 cols each.
                # q on the SP HWDGE ring, k on the ACT ring.
                qk2 = qk_pool.tile([C, 4 * P * E], F32)
                nc.sync.dma_start(
                    out=qk2[:, 0 : 2 * P * E],
                    in_=qr2[cb],
                )
                nc.scalar.dma_start(
                    out=qk2[:, 2 * P * E : 4 * P * E],
                    in_=kr2[cb],
                )
                # exp and relu for both chunks, bf16
                et2 = e_pool.tile([C, 4 * P * E], BF16)
                nc.scalar.activation(et2[:], qk2[:], AF.Exp)
                xm2 = x_pool.tile([C, 4 * P * E], BF16)
                nc.vector.tensor_scalar_max(xm2[:], qk2[:], 0.0)
            qof = c2 * P * E
            kof = 2 * P * E + c2 * P * E

            # ---- v with ones column, cast to bf16 during DMA (SWDGE)
            vx = vx_pool.tile([C, P * M1], BF16)
            vx3 = vx[:].rearrange("p (j m) -> p j m", j=P)
            nc.gpsimd.memset(vx3[:, :, E : E + 1], 1.0)
            nc.gpsimd.dma_start(out=vx3[:, :, 0:E], in_=vr[ci])

            # ---- phi = min(exp,1) + max(x,0) into the padded layout (bf16)
            phi = phi_bufs[ci % NBUF]
            # q even pairs -> blocks 0-1 low halves; q odd -> blocks 2-3
            # high halves; k -> cols 512:768 contiguous
            pq = phi[:].rearrange("p (a b f) -> p a b f", b=2, f=64)
            eq = et2[:, qof : qof + P * E].rearrange(
                "p (a b f) -> p a b f", b=2, f=64
            )
            xq = xm2[:, qof : qof + P * E].rearrange(
                "p (a b f) -> p a b f", b=2, f=64
            )
            nc.vector.scalar_tensor_tensor(
                pq[:, 0:2, 0, :], eq[:, 0:2, 0, :], 1.0, xq[:, 0:2, 0, :],
                op0=ALU.min, op1=ALU.add,
            )
            nc.vector.scalar_tensor_tensor(
                pq[:, 2:4, 1, :], eq[:, 0:2, 1, :], 1.0, xq[:, 0:2, 1, :],
                op0=ALU.min, op1=ALU.add,
            )
            nc.vector.scalar_tensor_tensor(
                phi[:, 512:768], et2[:, kof : kof + P * E], 1.0,
                xm2[:, kof : kof + P * E], op0=ALU.min, op1=ALU.add,
            )

            # ---- transpose the 6 blocks to E-major via PE identity matmuls
            # (out = phi_block^T @ I); the HWDGE rings can't absorb xbar
            # transposes (1.2us each, FIFO per ring)
            tq_ps = ps_tq.tile([C, 4 * C], F32)
            tk_ps = ps_tk.tile([C, 2 * C], F32)
            for b in range(4):
                nc.tensor.matmul(
                    tq_ps[:, b * C : (b + 1) * C],
                    phi[:, b * C : (b + 1) * C],
                    ident[:],
                    start=(b == 0),
                    stop=(b == 3),
                )
            for b in range(2):
                nc.tensor.matmul(
                    tk_ps[:, b * C : (b + 1) * C],
                    phi[:, (4 + b) * C : (5 + b) * C],
                    ident[:],
                    start=(b == 0),
                    stop=(b == 1),
                )
            qkT = qkT_pool.tile([C, 6 * C], BF16)
            nc.scalar.activation(qkT[:, 0 : 4 * C], tq_ps[:], AF.Copy)
            nc.vector.tensor_copy(qkT[:, 4 * C : 6 * C], tk_ps[:])

            def qT(j):
                b = _QBLK[j]
                return qkT[:, b * C : (b + 1) * C]

            def kT(j):
                b = 4 + j // 2
                return qkT[:, b * C : (b + 1) * C]

            # ---- attn_T[d, c] = K_d . Q_c  (one PSUM bank for all 4 pairs)
            attn_ps = ps_attn.tile([C, P * C], F32)
            for j in range(P):
                nc.tensor.matmul(
                    attn_ps[:, j * C : (j + 1) * C],
                    kT(j),
                    qT(j),
                    start=(j == 0),
                    stop=(j == P - 1),
                )

            # ---- causal mask (upper-tri in [d, c]) * key_lengths[d], to bf16
            attn_sb = attn_pool.tile([C, P * C], BF16)
            nc.vector.scalar_tensor_tensor(
                attn_sb[:], attn_ps[:], kl_t[:, ci : ci + 1], tril_t[:],
                op0=ALU.mult, op1=ALU.mult,
            )

            # ---- out = attn_T^T @ V' (+ Q @ S_prev)
            out_ps = ps_out.tile([C, P * M1], F32)
            for j in range(P):
                nc.tensor.matmul(
                    out_ps[:, j * M1 : (j + 1) * M1],
                    attn_sb[:, j * C : (j + 1) * C],
                    vx[:, j * M1 : (j + 1) * M1],
                    start=(j == 0),
                    stop=(ci == 0 and j == P - 1),
                )
            if ci > 0:
                for j in range(P):
                    nc.tensor.matmul(
                        out_ps[:, j * M1 : (j + 1) * M1],
                        qT(j),
                        s_prev[:, (j // 2) * M1 : (j // 2 + 1) * M1],
                        start=False,
                        stop=(j == P - 1),
                    )

            # ---- S += K^T @ V' (accumulates in PSUM across all chunks)
            for j in range(P):
                lo = 64 * (j % 2)
                nc.tensor.matmul(
                    s_psum[lo : lo + 64, (j // 2) * M1 : (j // 2 + 1) * M1],
                    phi[:, 512 + j * E : 512 + (j + 1) * E],  # k_j [128, 64]
                    vx[:, j * M1 : (j + 1) * M1],
                    start=(ci == 0 and j <= 1),
                    stop=(ci == nch - 1 and j >= P - 2),
                    skip_group_check=True,
                )

            # ---- S -> SBUF (bf16) for next chunk's inter term
            if ci < nch - 1:
                s_sb = s_pool.tile([C, (P // 2) * M1], BF16)
                nc.scalar.activation(s_sb[:], s_psum[:, 0 : (P // 2) * M1], AF.Copy)
                s_prev = s_sb

            # ---- normalize: out[:, :64] * 1/denom  (denom = ones-column)
            out3 = out_ps[:].rearrange("p (j m) -> p j m", m=M1)
            zt = z_pool.tile([C, P], F32)
            nc.vector.reciprocal(zt[:], out3[:, :, E])
            if c2 == 0:
                osb2 = out_pool.tile([C, 2 * P * E], F32)
            nc.vector.tensor_mul(
                osb2[:, c2 * P * E : (c2 + 1) * P * E].rearrange(
                    "p (j e) -> p j e", j=P
                ),
                out3[:, :, 0:E],
                zt[:].unsqueeze(2).to_broadcast((C, P, E)),
            )
            if c2 == 1:
                nc.sync.dma_start(out=outr2[cb], in_=osb2[:])

    return nc


def _tril_mask():
    m = np.triu(np.ones((C, C), np.float32))  # keep d<=c in [d,c] layout
    return np.ascontiguousarray(np.tile(m, (1, P)))


def _ident_bf16():
    return np.eye(C, dtype=ml_dtypes.bfloat16)


_CACHE = {}


def _get_nc():
    if "nc" not in _CACHE:
        nc = bacc.Bacc("TRN2", target_bir_lowering=False, debug=False)
        build_core_kernel(nc)
        nc.compile()
        _CACHE["nc"] = nc
    return _CACHE["nc"]


def kernel(queries, keys, values, key_lengths):
    queries = np.asarray(queries, np.float32)
    keys = np.asarray(keys, np.float32)
    values = np.asarray(values, np.float32)
    key_lengths = np.asarray(key_lengths, np.float32)

    nc = _get_nc()
    tril = _tril_mask()
    in_maps = []
    for c in range(N_CORES):
        n, hg = c // 2, (c % 2) * P
        in_maps.append(
            {
                "q": np.ascontiguousarray(queries[n, :, hg : hg + P, :]),
                "k": np.ascontiguousarray(keys[n, :, hg : hg + P, :]),
                "v": np.ascontiguousarray(values[n, :, hg : hg + P, :]),
                "kl": np.ascontiguousarray(key_lengths[n]),
                "tril": tril,
                "ident": _ident_bf16(),
            }
        )
    res = run_bass_kernel_spmd(nc, in_maps, list(range(N_CORES)))
    out = np.empty((N, L, H, E), np.float32)
    for c, r in enumerate(res.results):
        n, hg = c // 2, (c % 2) * P
        out[n, :, hg : hg + P, :] = r["out"]
    return out


# revision 29
# speedup vs baseline: 4.0478x; 1.0502x over previous
"""Causal linear attention (fast-transformers style) on 8 Trainium2 NeuronCores.

Full inputs in, full output out. Sharding: the 32 (n, h) pairs are split
8 ways -> each core owns 4 pairs (one batch n, 4 adjacent heads), so the
per-(n,h) cumulative KV state never crosses cores (no collectives).

Per-core algorithm (chunked scan, chunk C=128 rows):
  phi(x) = elu(x)+1 = max(x,0) + min(exp(x),1)      (exact identity)
  Q = phi(q); K = phi(k) * kl;  V' = [V, 1]          (ones column produces
                                                      the denominator)
  per chunk i, per pair j:
    attn_T[d,c] = sum_e K[d,e] Q[c,e]   masked to d<=c (triu in [d,c])
    out = attn_T^T @ V' + Q @ S         (S = running sum of K^T V' [E, M+1])
    S  += K^T @ V'                      (PSUM accumulation across all chunks)
    result = out[:, :64] / out[:, 64]   (phi>0 so no eps needed; |eps/denom|
                                         would be ~1e-8)

Matmul operands are bf16 (PSUM accumulation stays fp32). Q/K are
transposed to E-major via the DMA xbar. All matmul operands are padded to
K=128 at partition base 0 (zero half-blocks kill the cross terms): this
toolchain's PE crashes (NRT_EXEC_UNIT_UNRECOVERABLE) when consecutive
matmuls' operand base partitions alternate between 0 and 64.

Transposed-block layout (128 cols each) of the phi tile:
  [q0|Z] [q2|Z] [Z|q1] [Z|q3] [k0|k1] [k2|k3]
so qT_j lands on the partition half matching pair parity, kT blocks carry
two pairs. The S state mirrors that parity: pair j at partitions
64*(j%2).., cols 65*(j//2)..
"""

from contextlib import ExitStack

import ml_dtypes
import numpy as np

import concourse.bacc as bacc
import concourse.mybir as mybir
import concourse.tile as tile
from concourse.bass_utils import run_bass_kernel_spmd

F32 = mybir.dt.float32
BF16 = mybir.dt.bfloat16
AF = mybir.ActivationFunctionType
ALU = mybir.AluOpType

N, L, H, E = 4, 4096, 8, 64
P = 4            # (n,h) pairs per core
C = 128          # chunk rows
M1 = E + 1       # v columns + ones column (denominator)
N_CORES = 8
NBUF = 3         # phi ring depth

# col offset of each pair's q data inside the padded q-block region
_QCOL = {0: 0, 2: 128, 1: 320, 3: 448}
# block index of each pair's padded qT block
_QBLK = {0: 0, 2: 1, 1: 2, 3: 3}


def build_core_kernel(nc, seq_len=L):
    """Emit the per-core program. Each core sees [seq_len, P, E] slices."""
    nch = seq_len // C

    q_d = nc.dram_tensor("q", [seq_len, P, E], F32, kind="ExternalInput").ap()
    k_d = nc.dram_tensor("k", [seq_len, P, E], F32, kind="ExternalInput").ap()
    v_d = nc.dram_tensor("v", [seq_len, P, E], F32, kind="ExternalInput").ap()
    kl_d = nc.dram_tensor("kl", [seq_len], F32, kind="ExternalInput").ap()
    tril_d = nc.dram_tensor("tril", [C, P * C], F32, kind="ExternalInput").ap()
    ident_d = nc.dram_tensor("ident", [C, C], BF16, kind="ExternalInput").ap()
    out_d = nc.dram_tensor("out", [seq_len, P, E], F32, kind="ExternalOutput").ap()

    qr = q_d.rearrange("(c p) j e -> c p (j e)", p=C)
    kr = k_d.rearrange("(c p) j e -> c p (j e)", p=C)
    qr2 = q_d.rearrange("(c t p) j e -> c p t (j e)", t=2, p=C)
    kr2 = k_d.rearrange("(c t p) j e -> c p t (j e)", t=2, p=C)
    vr = v_d.rearrange("(c p) j e -> c p j e", p=C)
    klr = kl_d.rearrange("(c p) -> p c", p=C)
    outr = out_d.rearrange("(c p) j e -> c p (j e)", p=C)
    outr2 = out_d.rearrange("(c t p) j e -> c p t (j e)", t=2, p=C)

    with tile.TileContext(nc) as tc, ExitStack() as ctx:
        consts = ctx.enter_context(tc.tile_pool(name="consts", bufs=1))
        qk_pool = ctx.enter_context(tc.tile_pool(name="qk", bufs=NBUF))
        e_pool = ctx.enter_context(tc.tile_pool(name="exp", bufs=2))
        x_pool = ctx.enter_context(tc.tile_pool(name="xmax", bufs=2))
        phi_pool = ctx.enter_context(tc.tile_pool(name="phi", bufs=1))
        vx_pool = ctx.enter_context(tc.tile_pool(name="vx", bufs=1))
        vf_pool = ctx.enter_context(tc.tile_pool(name="vf", bufs=NBUF))
        qkT_pool = ctx.enter_context(tc.tile_pool(name="qkT", bufs=NBUF))
        attn_pool = ctx.enter_context(tc.tile_pool(name="attn", bufs=NBUF))
        s_pool = ctx.enter_context(tc.tile_pool(name="ssb", bufs=2))
        z_pool = ctx.enter_context(tc.tile_pool(name="z", bufs=2))
        out_pool = ctx.enter_context(tc.tile_pool(name="osb", bufs=NBUF))
        ps_attn = ctx.enter_context(tc.tile_pool(name="psA", bufs=1, space="PSUM"))
        ps_out = ctx.enter_context(tc.tile_pool(name="psO", bufs=2, space="PSUM"))
        ps_s = ctx.enter_context(tc.tile_pool(name="psS", bufs=1, space="PSUM"))
        ps_tq = ctx.enter_context(tc.tile_pool(name="psTq", bufs=2, space="PSUM"))
        ps_tk = ctx.enter_context(tc.tile_pool(name="psTk", bufs=2, space="PSUM"))

        tril_t = consts.tile([C, P * C], F32)
        nc.sync.dma_start(out=tril_t[:], in_=tril_d[:])
        kl_t = consts.tile([C, nch], F32)
        nc.sync.dma_start(out=kl_t[:], in_=klr)
        ident = consts.tile([C, C], BF16)
        nc.sync.dma_start(out=ident[:], in_=ident_d[:])

        # persistent phi ring: [q0|Z][q2|Z][Z|q1][Z|q3][k0|k1][k2|k3], bf16.
        # The Z half-blocks are zeroed once and never written again.
        phi_bufs = []
        for i in range(NBUF):
            pb = phi_pool.tile([C, 6 * C], BF16, name=f"phib{i}")
            pb3 = pb[:].rearrange("p (b z e) -> p b z e", b=6, z=2)
            nc.gpsimd.memset(pb3[:, 0:2, 1, :], 0.0)  # blocks 0-1 high half
            nc.gpsimd.memset(pb3[:, 2:4, 0, :], 0.0)  # blocks 2-3 low half
            phi_bufs.append(pb)

        # persistent vx ring: [v_j | 1] per pair; ones columns preset once
        vx_bufs = []
        for i in range(NBUF):
            vb = vx_pool.tile([C, P * M1], BF16, name=f"vxb{i}")
            nc.gpsimd.memset(
                vb[:].rearrange("p (j m) -> p j m", j=P)[:, :, E : E + 1], 1.0
            )
            vx_bufs.append(vb)

        # running K^T V' state; pair j at partitions 64*(j%2).., cols
        # 65*(j//2)... Full 512-col row (one bank) keeps partition-offset
        # slices 2KB-aligned for the accumulate bookkeeping.
        s_psum = ps_s.tile([C, 512], F32)

        assert nch % 2 == 0
        s_prev = None
        for ci in range(nch):
            cb, c2 = ci // 2, ci % 2
            if c2 == 0:
                # ---- load q,k for TWO chunks (fp32): layout
                # [q(2c) | q(2c+1) | k(2c) | k(2c+1)], 256 cols each.
                # q on the SP HWDGE ring, k on the ACT ring.
                qk2 = qk_pool.tile([C, 4 * P * E], F32)
                nc.sync.dma_start(
                    out=qk2[:, 0 : 2 * P * E],
                    in_=qr2[cb],
                )
                nc.scalar.dma_start(
                    out=qk2[:, 2 * P * E : 4 * P * E],
                    in_=kr2[cb],
                )
                # exp and relu for both chunks, bf16
                et2 = e_pool.tile([C, 4 * P * E], BF16)
                nc.scalar.activation(et2[:], qk2[:], AF.Exp)
                xm2 = x_pool.tile([C, 4 * P * E], BF16)
                nc.vector.tensor_scalar_max(xm2[:], qk2[:], 0.0)
            qof = c2 * P * E
            kof = 2 * P * E + c2 * P * E

            # ---- v: fp32 load on the SP ring, DVE cast into the
            # persistent [v | 1] tile (ones preset once)
            vf = vf_pool.tile([C, P * E], F32)
            nc.sync.dma_start(out=vf[:], in_=vr[ci])
            vx = vx_bufs[ci % NBUF]
            vx3 = vx[:].rearrange("p (j m) -> p j m", j=P)
            nc.vector.tensor_copy(
                vx3[:, :, 0:E], vf[:].rearrange("p (j e) -> p j e", j=P)
            )

            # ---- phi = min(exp,1) + max(x,0) into the padded layout (bf16)
            phi = phi_bufs[ci % NBUF]
            # q even pairs -> blocks 0-1 low halves; q odd -> blocks 2-3
            # high halves; k -> cols 512:768 contiguous
            pq = phi[:].rearrange("p (a b f) -> p a b f", b=2, f=64)
            eq = et2[:, qof : qof + P * E].rearrange(
                "p (a b f) -> p a b f", b=2, f=64
            )
            xq = xm2[:, qof : qof + P * E].rearrange(
                "p (a b f) -> p a b f", b=2, f=64
            )
            nc.vector.scalar_tensor_tensor(
                pq[:, 0:2, 0, :], eq[:, 0:2, 0, :], 1.0, xq[:, 0:2, 0, :],
                op0=ALU.min, op1=ALU.add,
            )
            nc.vector.scalar_tensor_tensor(
                pq[:, 2:4, 1, :], eq[:, 0:2, 1, :], 1.0, xq[:, 0:2, 1, :],
                op0=ALU.min, op1=ALU.add,
            )
            nc.vector.scalar_tensor_tensor(
                phi[:, 512:768], et2[:, kof : kof + P * E], 1.0,
                xm2[:, kof : kof + P * E], op0=ALU.min, op1=ALU.add,
            )

            # ---- transpose the 6 blocks to E-major via PE identity matmuls
            # (out = phi_block^T @ I); the HWDGE rings can't absorb xbar
            # transposes (1.2us each, FIFO per ring)
            tq_ps = ps_tq.tile([C, 4 * C], F32)
            tk_ps = ps_tk.tile([C, 2 * C], F32)
            for b in range(4):
                nc.tensor.matmul(
                    tq_ps[:, b * C : (b + 1) * C],
                    phi[:, b * C : (b + 1) * C],
                    ident[:],
                    start=(b == 0),
                    stop=(b == 3),
                )
            for b in range(2):
                nc.tensor.matmul(
                    tk_ps[:, b * C : (b + 1) * C],
                    phi[:, (4 + b) * C : (5 + b) * C],
                    ident[:],
                    start=(b == 0),
                    stop=(b == 1),
                )
            qkT = qkT_pool.tile([C, 6 * C], BF16)
            nc.scalar.activation(qkT[:, 0 : 4 * C], tq_ps[:], AF.Copy)
            nc.scalar.activation(qkT[:, 4 * C : 6 * C], tk_ps[:], AF.Copy)

            def qT(j):
                b = _QBLK[j]
                return qkT[:, b * C : (b + 1) * C]

            def kT(j):
                b = 4 + j // 2
                return qkT[:, b * C : (b + 1) * C]

            # ---- attn_T[d, c] = K_d . Q_c  (one PSUM bank for all 4 pairs)
            attn_ps = ps_attn.tile([C, P * C], F32)
            for j in range(P):
                nc.tensor.matmul(
                    attn_ps[:, j * C : (j + 1) * C],
                    kT(j),
                    qT(j),
                    start=(j == 0),
                    stop=(j == P - 1),
                )

            # ---- causal mask (upper-tri in [d, c]) * key_lengths[d], to bf16
            attn_sb = attn_pool.tile([C, P * C], BF16)
            nc.vector.scalar_tensor_tensor(
                attn_sb[:], attn_ps[:], kl_t[:, ci : ci + 1], tril_t[:],
                op0=ALU.mult, op1=ALU.mult,
            )

            # ---- out = attn_T^T @ V' (+ Q @ S_prev)
            out_ps = ps_out.tile([C, P * M1], F32)
            for j in range(P):
                nc.tensor.matmul(
                    out_ps[:, j * M1 : (j + 1) * M1],
                    attn_sb[:, j * C : (j + 1) * C],
                    vx[:, j * M1 : (j + 1) * M1],
                    start=(j == 0),
                    stop=(ci == 0 and j == P - 1),
                )
            if ci > 0:
                for j in range(P):
                    nc.tensor.matmul(
                        out_ps[:, j * M1 : (j + 1) * M1],
                        qT(j),
                        s_prev[:, (j // 2) * M1 : (j // 2 + 1) * M1],
                        start=False,
                        stop=(j == P - 1),
                    )

            # ---- S += K^T @ V' (accumulates in PSUM across all chunks)
            for j in range(P):
                lo = 64 * (j % 2)
                nc.tensor.matmul(
                    s_psum[lo : lo + 64, (j // 2) * M1 : (j // 2 + 1) * M1],
                    phi[:, 512 + j * E : 512 + (j + 1) * E],  # k_j [128, 64]
                    vx[:, j * M1 : (j + 1) * M1],
                    start=(ci == 0 and j <= 1),
                    stop=(ci == nch - 1 and j >= P - 2),
                    skip_group_check=True,
                )

            # ---- S -> SBUF (bf16) for next chunk's inter term
            if ci < nch - 1:
                s_sb = s_pool.tile([C, (P // 2) * M1], BF16)
                nc.scalar.activation(s_sb[:], s_psum[:, 0 : (P // 2) * M1], AF.Copy)
                s_prev = s_sb

            # ---- normalize: out[:, :64] * 1/denom  (denom = ones-column)
            out3 = out_ps[:].rearrange("p (j m) -> p j m", m=M1)
            zt = z_pool.tile([C, P], F32)
            nc.vector.reciprocal(zt[:], out3[:, :, E])
            if c2 == 0:
                osb2 = out_pool.tile([C, 2 * P * E], F32)
            nc.vector.tensor_mul(
                osb2[:, c2 * P * E : (c2 + 1) * P * E].rearrange(
                    "p (j e) -> p j e", j=P
                ),
                out3[:, :, 0:E],
                zt[:].unsqueeze(2).to_broadcast((C, P, E)),
            )
            if c2 == 1:
                nc.sync.dma_start(out=outr2[cb], in_=osb2[:])

    return nc


def _tril_mask():
    m = np.triu(np.ones((C, C), np.float32))  # keep d<=c in [d,c] layout
    return np.ascontiguousarray(np.tile(m, (1, P)))


def _ident_bf16():
    return np.eye(C, dtype=ml_dtypes.bfloat16)


_CACHE = {}


def _get_nc():
    if "nc" not in _CACHE:
        nc = bacc.Bacc("TRN2", target_bir_lowering=False, debug=False)
        build_core_kernel(nc)
        nc.compile()
        _CACHE["nc"] = nc
    return _CACHE["nc"]


def kernel(queries, keys, values, key_lengths):
    queries = np.asarray(queries, np.float32)
    keys = np.asarray(keys, np.float32)
    values = np.asarray(values, np.float32)
    key_lengths = np.asarray(key_lengths, np.float32)

    nc = _get_nc()
    tril = _tril_mask()
    in_maps = []
    for c in range(N_CORES):
        n, hg = c // 2, (c % 2) * P
        in_maps.append(
            {
                "q": np.ascontiguousarray(queries[n, :, hg : hg + P, :]),
                "k": np.ascontiguousarray(keys[n, :, hg : hg + P, :]),
                "v": np.ascontiguousarray(values[n, :, hg : hg + P, :]),
                "kl": np.ascontiguousarray(key_lengths[n]),
                "tril": tril,
                "ident": _ident_bf16(),
            }
        )
    res = run_bass_kernel_spmd(nc, in_maps, list(range(N_CORES)))
    out = np.empty((N, L, H, E), np.float32)
    for c, r in enumerate(res.results):
        n, hg = c // 2, (c % 2) * P
        out[n, :, hg : hg + P, :] = r["out"]
    return out
